# revision 82
# baseline (speedup 1.0000x reference)
"""Trainium2 Bass kernel for CoherentDONN (3-layer diffractive optical NN).

Math: per layer, field update is
    U' = ifft2(H * fft2(U * exp(i*phi_l)))
H is separable (H = e^{ikz} h x h, Fresnel chirp), so the whole linear step
collapses to  U' = A V A^T  with  A = conj(F) diag(h) F / 512  (circulant,
unitary).  e^{ikz} is unit-modulus and drops out of the final intensity.
On the PE (out = lhsT.T @ rhs) with W = A^T:
    S = V^T W      (= (A V)^T)
    U' = S^T W     (= A V A^T)
so each layer is exactly two 512-contraction complex matmul stages with the
same moving operand W and zero transposes/elementwise H work.

Each complex matmul stage uses the 3M (Karatsuba) decomposition:
    P1 = Vre Wre,  P2 = Vim Wim,  P3 = (Vre+Vim)(Wre+Wim)
    Sre = P1 - P2,  Sim = P3 - P1 - P2
i.e. 12 accumulating 128x128x512 matmuls per 128-row output block instead of
the schoolbook 16 (bf16 operands run the PE at full rate and rel-err stays
~1.2e-2, well inside the 2e-2 gate).

Engine layout (engines read at most ONE PSUM operand per instruction): the
Act engine drains each PSUM product to SBUF (each copy hides under the next
matmul chain, so PSUM ring slots free ~1us after their chain); the DVE does
all combines/phase-rotation as all-16-bit SBUF ops in its 2x mode; Pool
(gpsimd) takes slack-tolerant plane sums; layer-0's phase multiply is folded
into the host-prepared input planes.  Two images are kept in flight so every
producer has a full stage of latency cover; the vim/vsum half of each phase
rotation is emitted one stage deferred to keep the DVE inside the layer-
boundary window budget.  TimelineSim makespan: ~1.028 ms/core (PE ~97% busy
at the 12-matmul/block floor).

Sharding: pure data parallelism, 16 images per core across 8 cores.
"""

import os
import numpy as np

import concourse.bass as bass
import concourse.mybir as mybir
import concourse.tile as tile
from concourse import bacc
from concourse.bass_utils import run_bass_kernel_spmd

N_CORES = int(os.environ.get("DONN_CORES", "8"))
PER_CORE = int(os.environ.get("DONN_IMG", str(128 // max(N_CORES, 1))))
RES = 512
NL = 3
NCLS = 10
RB = RES // 128            # 4 row blocks of 128 partitions
FDIM = RES * RES // 128    # 2048 feat positions per partition
FC_BLK = 256               # feat positions per FC chunk
LAMBDA = 5.32e-07
Z = 0.035
DX = 1e-06

ALG = os.environ.get("DONN_ALG", "3m")   # "3m" | "school"

f32 = mybir.dt.float32
f32r = mybir.dt.float32r
bf16 = mybir.dt.bfloat16
_mdt_default = "bf16" if ALG == "3m" else "f32r"
MDT = {"f32r": f32r, "bf16": bf16, "f32": f32}[
    os.environ.get("DONN_MMDT", _mdt_default)]
MULT = mybir.AluOpType.mult
ADD = mybir.AluOpType.add
SUB = mybir.AluOpType.subtract
SQUARE = mybir.ActivationFunctionType.Square
COPY = mybir.ActivationFunctionType.Copy


def _host_constants():
    fx = np.fft.fftfreq(RES, DX)
    h = np.exp(-1j * np.pi * LAMBDA * Z * fx**2)
    a = np.fft.ifft(h)
    idx = (np.arange(RES)[:, None] - np.arange(RES)[None, :]) % RES
    W = a[idx].T.copy()  # W = A^T, complex128
    # [row k, col n] -> [p, c, n] with k = c*128 + p
    def lay(m):
        return np.ascontiguousarray(
            m.reshape(RB, 128, RES).transpose(1, 0, 2).astype(np.float32)
        )
    return (lay(W.real), lay(W.imag), lay(-W.imag), lay(W.real + W.imag))


def _build(nc_handle_cache={}):
    if "nc" in nc_handle_cache:
        return nc_handle_cache["nc"], nc_handle_cache["aps"]

    nc = bacc.Bacc("TRN2", target_bir_lowering=False, debug=False,
                   num_devices=N_CORES)

    # layer-0 phase multiply is precomputed on host: per image the planes
    # (x*cos(phi0), x*sin(phi0)[, x*(cos+sin)]) ship as MDT directly.
    NV = 3 if ALG == "3m" else 2
    x_d = nc.dram_tensor("v0", [PER_CORE, NV, 128, RB, RES], MDT,
                         kind="ExternalInput").ap()
    PDT = f32 if os.environ.get("DONN_PMDT") == "f32" else MDT
    wre_d = nc.dram_tensor("wre", [128, RB, RES], MDT, kind="ExternalInput").ap()
    wim_d = nc.dram_tensor("wim", [128, RB, RES], MDT, kind="ExternalInput").ap()
    if ALG == "3m":
        wx_d = nc.dram_tensor("wsum", [128, RB, RES], MDT, kind="ExternalInput").ap()
    else:
        wx_d = nc.dram_tensor("wimn", [128, RB, RES], MDT, kind="ExternalInput").ap()
    pc_d = nc.dram_tensor("pcos", [NL - 1, 128, RB, RES], PDT,
                          kind="ExternalInput").ap()
    psn_d = nc.dram_tensor("psin", [NL - 1, 128, RB, RES], PDT,
                           kind="ExternalInput").ap()
    fcw_d = nc.dram_tensor("fcw", [128, NCLS, FDIM], bf16, kind="ExternalInput").ap()
    fcb_d = nc.dram_tensor("fcb", [PER_CORE, NCLS], f32, kind="ExternalInput").ap()
    out_d = nc.dram_tensor("out", [PER_CORE, NCLS], f32, kind="ExternalOutput").ap()

    with tile.TileContext(nc) as tc:
        with tc.tile_pool(name="consts", bufs=1) as constp, \
             tc.tile_pool(name="dram", bufs=1, space="DRAM") as dramp:
            featbuf = dramp.tile([PER_CORE, 128, FDIM], bf16)

            # W planes and phase masks arrive pre-converted to MDT/PDT from
            # the host: no staging, no convert, minimal startup DMA.
            w_tiles = {}
            for name, src in (("wre", wre_d), ("wim", wim_d), ("wx", wx_d)):
                wt = constp.tile([128, RB, RES], MDT, tag=name)
                nc.sync.dma_start(wt[:], src[:])
                w_tiles[name] = wt
            wre, wim, wx = w_tiles["wre"], w_tiles["wim"], w_tiles["wx"]

            pcos, psin = [None], [None]
            for l in range(1, NL):
                for src, lst, nm in ((pc_d, pcos, "pc"), (psn_d, psin, "ps")):
                    ct = constp.tile([128, RB, RES], PDT, tag=f"{nm}{l}")
                    nc.sync.dma_start(ct[:], src[l - 1])
                    lst.append(ct)

            fcb_t = constp.tile([PER_CORE, NCLS], f32, tag="fcb")
            nc.sync.dma_start(fcb_t[:], fcb_d[:])

            with tc.tile_pool(name="vp", bufs=4) as vpool, \
                 tc.tile_pool(name="up", bufs=4) as upool, \
                 tc.tile_pool(name="sp", bufs=2) as spool, \
                 tc.tile_pool(name="tp", bufs=3) as tpool, \
                 tc.tile_pool(name="fp", bufs=3) as fpool, \
                 tc.tile_pool(name="ps", bufs=4 if ALG == "3m" else 8,
                              space="PSUM") as psum:

                # ---------------- 3M (Karatsuba) path ----------------
                def load_and_pm1_3m(i):
                    # scalar-engine DMA queue: image loads bypass the sync
                    # queue that carries W staging and feature writebacks
                    vre = vpool.tile([128, RB, RES], MDT, tag="vre")
                    vim = vpool.tile([128, RB, RES], MDT, tag="vim")
                    vsm = vpool.tile([128, RB, RES], MDT, tag="vsm")
                    nc.scalar.dma_start(vre[:], x_d[i, 0])
                    nc.scalar.dma_start(vim[:], x_d[i, 1])
                    nc.scalar.dma_start(vsm[:], x_d[i, 2])
                    return vre, vim, vsm

                def mm_stage_3m(lre, lim, lsm, to_sbuf):
                    """One complex stage via 3M.  PSUM tiles span TWO row
                    blocks (2 banks); the Act engine drains each product to
                    SBUF and the DVE combines the SBUF copies.  If to_sbuf,
                    produce MDT planes (sre, sim, ssum) for the next stage;
                    else produce per-pair (ure, uim) for phase rotation or
                    intensity."""
                    if to_sbuf:
                        sre = spool.tile([128, RB, RES], MDT, tag="sre")
                        sim = spool.tile([128, RB, RES], MDT, tag="sim")
                        ssm = spool.tile([128, RB, RES], MDT, tag="ssm")

                    def chain(ptile, h, mi, lhs, w):
                        ms = bass.ts(mi, 128)
                        for c in range(RB):
                            nc.tensor.matmul(ptile[:, h, :], lhs[:, c, ms],
                                             w[:, c, :],
                                             start=(c == 0), stop=(c == RB - 1))

                    # Two row-block pairs share the 8-bank ring (4 tiles x 2
                    # banks).  Each product's Act copy is its only PSUM
                    # reader and hides under the next matmul chain, so ring
                    # slots free ~1us after their chain and the PE never
                    # waits on a PSUM slot.
                    # Engines read at most ONE PSUM input per instruction,
                    # so each product is drained psum->SBUF by the idle Act
                    # engine (every copy hides under the next matmul chain
                    # and frees its ring slot ~1us after the chain ends).
                    # All combines then run as all-16-bit SBUF DVE ops at
                    # the 2x rate, fully decoupled from the PSUM ring.
                    out = [None] * (RB // 2)
                    for mp in range(RB // 2):
                        p1 = psum.tile([128, 2, RES], f32, tag="st", name="p1")
                        for h in range(2):
                            chain(p1, h, 2 * mp + h, lre, wre)
                        cp1 = tpool.tile([128, 2, RES], MDT, tag="cp1")
                        nc.scalar.activation(cp1[:], p1[:], COPY)
                        p2 = psum.tile([128, 2, RES], f32, tag="st", name="p2")
                        for h in range(2):
                            chain(p2, h, 2 * mp + h, lim, wim)
                        cp2 = tpool.tile([128, 2, RES], MDT, tag="cp2")
                        nc.scalar.activation(cp2[:], p2[:], COPY)
                        mslc = bass.ts(mp, 2)
                        u = tpool.tile([128, 2, RES], MDT, tag="u", name="u")
                        if to_sbuf:
                            nc.vector.tensor_tensor(sre[:, mslc, :], cp1[:],
                                                    cp2[:], SUB)
                        else:
                            ure = upool.tile([128, 2, RES], MDT, tag="ure")
                            nc.vector.tensor_tensor(ure[:], cp1[:], cp2[:], SUB)
                        nc.vector.tensor_tensor(u[:], cp1[:], cp2[:], ADD)
                        p3 = psum.tile([128, 2, RES], f32, tag="st", name="p3")
                        for h in range(2):
                            chain(p3, h, 2 * mp + h, lsm, wx)
                        cp3 = tpool.tile([128, 2, RES], MDT, tag="cp3")
                        nc.scalar.activation(cp3[:], p3[:], COPY)
                        if to_sbuf:
                            nc.vector.tensor_tensor(sim[:, mslc, :], cp3[:],
                                                    u[:], SUB)
                            # ssm's consumer (next stage's P3 chain) is ~3
                            # chains away: Pool's latency is fine, and this
                            # keeps the DVE queue free of head-of-line work
                            nc.gpsimd.tensor_tensor(ssm[:, mslc, :],
                                                    sre[:, mslc, :],
                                                    sim[:, mslc, :], ADD)
                        else:
                            uim = upool.tile([128, 2, RES], MDT, tag="uim")
                            nc.vector.tensor_tensor(uim[:], cp3[:], u[:], SUB)
                            out[mp] = (ure, uim)
                    if to_sbuf:
                        return sre, sim, ssm
                    return out

                def pm_from_psum_3m(l, us):
                    """V_l = U_{l-1} * exp(i*phi_l) from per-pair (ure, uim).

                    The vre path (needed by the next stage's first chains)
                    is emitted inline on the DVE.  The vim/vsm path is
                    returned as a deferred closure: it's consumed two stage
                    slots later, so the caller emits it after the paired
                    image's stage — keeping the DVE queue inside the layer-
                    boundary window budget."""
                    vre = vpool.tile([128, RB, RES], MDT, tag="vre")
                    vim = vpool.tile([128, RB, RES], MDT, tag="vim")
                    vsm = vpool.tile([128, RB, RES], MDT, tag="vsm")
                    for mp, (ure, uim) in enumerate(us):
                        mslc = bass.ts(mp, 2)
                        c_ap = pcos[l][:, mslc, :]
                        s_ap = psin[l][:, mslc, :]
                        t1 = upool.tile([128, 2, RES], MDT, tag="t1")
                        t2 = upool.tile([128, 2, RES], MDT, tag="t2")
                        nc.vector.tensor_tensor(t1[:], ure[:], c_ap, MULT)
                        nc.vector.tensor_tensor(t2[:], uim[:], s_ap, MULT)
                        nc.vector.tensor_tensor(vre[:, mslc, :], t1[:], t2[:], SUB)

                    def finish_vim():
                        for mp, (ure, uim) in enumerate(us):
                            mslc = bass.ts(mp, 2)
                            c_ap = pcos[l][:, mslc, :]
                            s_ap = psin[l][:, mslc, :]
                            t3 = upool.tile([128, 2, RES], MDT, tag="t3")
                            t4 = upool.tile([128, 2, RES], MDT, tag="t4")
                            nc.gpsimd.tensor_tensor(t3[:], ure[:], s_ap, MULT)
                            nc.vector.tensor_tensor(t4[:], uim[:], c_ap, MULT)
                            nc.vector.tensor_tensor(vim[:, mslc, :],
                                                    t3[:], t4[:], ADD)
                            nc.gpsimd.tensor_tensor(vsm[:, mslc, :],
                                                    vre[:, mslc, :],
                                                    vim[:, mslc, :], ADD)
                    return (vre, vim, vsm), finish_vim

                def intensity_3m(i, us):
                    # split squares across Act/DVE and keep the ft add on
                    # the DVE: this chain gates the FC readback at the very
                    # end of the kernel, so its serial latency matters
                    for mp, (ure, uim) in enumerate(us):
                        s0 = tpool.tile([128, 2, RES], bf16, tag="s0")
                        s1 = tpool.tile([128, 2, RES], bf16, tag="s1")
                        nc.scalar.activation(s0[:], ure[:], SQUARE)
                        nc.vector.tensor_tensor(s1[:], uim[:], uim[:], MULT)
                        ft = fpool.tile([128, 2, RES], bf16, tag="ft")
                        nc.vector.tensor_tensor(ft[:], s0[:], s1[:], ADD)
                        nc.sync.dma_start(
                            featbuf[i, :, bass.ts(mp, 2 * RES)], ft[:])

                # ---------------- schoolbook path (A/B reference) --------
                def load_and_pm1(i):
                    vre = vpool.tile([128, RB, RES], MDT, tag="vre")
                    vim = vpool.tile([128, RB, RES], MDT, tag="vim")
                    nc.sync.dma_start(vre[:], x_d[i, 0])
                    nc.sync.dma_start(vim[:], x_d[i, 1])
                    return vre, vim

                def mm_stage(lre, lim, to_sbuf):
                    if to_sbuf:
                        sre = spool.tile([128, RB, RES], MDT, tag="sre")
                        sim = spool.tile([128, RB, RES], MDT, tag="sim")
                    ps_pairs = []
                    for m in range(RB):
                        ms = bass.ts(m, 128)
                        pr = psum.tile([128, RES], f32, tag="st")
                        pi = psum.tile([128, RES], f32, tag="st")
                        for c in range(RB):
                            nc.tensor.matmul(pr[:], lre[:, c, ms], wre[:, c, :],
                                             start=(c == 0), stop=False)
                        for c in range(RB):
                            nc.tensor.matmul(pr[:], lim[:, c, ms], wx[:, c, :],
                                             start=False, stop=(c == RB - 1))
                        for c in range(RB):
                            nc.tensor.matmul(pi[:], lre[:, c, ms], wim[:, c, :],
                                             start=(c == 0), stop=False)
                        for c in range(RB):
                            nc.tensor.matmul(pi[:], lim[:, c, ms], wre[:, c, :],
                                             start=False, stop=(c == RB - 1))
                        if to_sbuf:
                            nc.vector.tensor_copy(sre[:, m, :], pr[:])
                            nc.scalar.activation(sim[:, m, :], pi[:], COPY)
                        else:
                            ps_pairs.append((pr, pi))
                    if to_sbuf:
                        return sre, sim
                    return ps_pairs

                def pm_from_psum(l, ps_pairs):
                    vre = vpool.tile([128, RB, RES], MDT, tag="vre")
                    vim = vpool.tile([128, RB, RES], MDT, tag="vim")
                    for m, (pr, pi) in enumerate(ps_pairs):
                        c_ap = pcos[l][:, m, :]
                        s_ap = psin[l][:, m, :]
                        t1 = tpool.tile([128, RES], f32, tag="t")
                        t2 = tpool.tile([128, RES], f32, tag="t")
                        nc.vector.tensor_tensor(t1[:], pr[:], c_ap, MULT)
                        nc.vector.tensor_tensor(t2[:], pi[:], s_ap, MULT)
                        nc.vector.tensor_tensor(vre[:, m, :], t1[:], t2[:], SUB)
                        t3 = tpool.tile([128, RES], f32, tag="t")
                        t4 = tpool.tile([128, RES], f32, tag="t")
                        nc.vector.tensor_tensor(t3[:], pr[:], s_ap, MULT)
                        nc.vector.tensor_tensor(t4[:], pi[:], c_ap, MULT)
                        nc.vector.tensor_tensor(vim[:, m, :], t3[:], t4[:], ADD)
                    return vre, vim

                def intensity(i, ps_pairs):
                    for m, (pr, pi) in enumerate(ps_pairs):
                        s0 = tpool.tile([128, RES], f32, tag="t")
                        s1 = tpool.tile([128, RES], f32, tag="t")
                        nc.scalar.activation(s0[:], pr[:], SQUARE)
                        nc.scalar.activation(s1[:], pi[:], SQUARE)
                        ft = fpool.tile([128, RES], bf16, tag="ft")
                        nc.vector.tensor_tensor(ft[:], s0[:], s1[:], ADD)
                        nc.sync.dma_start(featbuf[i, :, bass.ts(m, RES)], ft[:])

                if ALG == "3m":
                    f_load, f_stage, f_pm, f_int = (
                        load_and_pm1_3m, mm_stage_3m, pm_from_psum_3m,
                        intensity_3m)
                else:
                    f_load, f_stage, f_pm, f_int = (
                        load_and_pm1, mm_stage, pm_from_psum, intensity)

                npair = (PER_CORE + 1) // 2
                vcur = {}
                vcur[0] = f_load(0)
                if PER_CORE > 1:
                    vcur[1] = f_load(1)
                for pr_i in range(npair):
                    imgs = [i for i in (2 * pr_i, 2 * pr_i + 1) if i < PER_CORE]
                    for l in range(NL):
                        s_tiles = {}
                        for i in imgs:
                            s_tiles[i] = f_stage(*vcur[i], to_sbuf=True)
                        deferred = []
                        for i in imgs:
                            ps = f_stage(*s_tiles[i], to_sbuf=False)
                            if l < NL - 1:
                                if ALG == "3m":
                                    vcur[i], fin = f_pm(l + 1, ps)
                                    deferred.append(fin)
                                else:
                                    vcur[i] = f_pm(l + 1, ps)
                            else:
                                f_int(i, ps)
                        for fin in deferred:
                            fin()
                        if l == 0:
                            for i_next in (2 * pr_i + 2, 2 * pr_i + 3):
                                if i_next < PER_CORE:
                                    vcur[i_next] = f_load(i_next)

            # ---- FC over all images ----
            # wch chunks are static weights: preload them all on a separate
            # queue so the tail is only fch readback + matmul.
            nblk = FDIM // FC_BLK
            with tc.tile_pool(name="fcw", bufs=nblk) as fwpool, \
                 tc.tile_pool(name="fcp", bufs=8) as fcpool, \
                 tc.tile_pool(name="fps", bufs=1, space="PSUM") as fpsum:
                wchs = []
                for blk in range(nblk):
                    wch = fwpool.tile([128, NCLS, FC_BLK], bf16, tag="wch")
                    nc.gpsimd.dma_start(wch[:], fcw_d[:, :, bass.ts(blk, FC_BLK)])
                    wchs.append(wch)
                ps_fc = fpsum.tile([PER_CORE, NCLS], f32, tag="fc")
                feat_t = featbuf[:].rearrange("i p f -> p i f")
                fc_q = [nc.sync, nc.scalar, nc.gpsimd]
                for blk in range(nblk):
                    fs = bass.ts(blk, FC_BLK)
                    fch = fcpool.tile([128, PER_CORE, FC_BLK], bf16, tag="fch")
                    # rotate DMA queues so chunk readbacks overlap
                    fc_q[blk % 3].dma_start(fch[:], feat_t[:, :, fs])
                    for j in range(FC_BLK):
                        nc.tensor.matmul(ps_fc[:], fch[:, :, j], wchs[blk][:, :, j],
                                         start=(blk == 0 and j == 0),
                                         stop=(blk == nblk - 1 and j == FC_BLK - 1))
                out_sb = fcpool.tile([PER_CORE, NCLS], f32, tag="osb")
                nc.vector.tensor_tensor(out_sb[:], ps_fc[:], fcb_t[:], ADD)
                nc.sync.dma_start(out_d[:], out_sb[:])

    nc.compile()
    aps = None
    nc_handle_cache["nc"] = nc
    nc_handle_cache["aps"] = aps
    return nc, aps


def kernel(x, phases, fc_w, fc_b):
    x = np.asarray(x, dtype=np.float32)
    phases = np.asarray(phases, dtype=np.float32)
    fc_w = np.asarray(fc_w, dtype=np.float32)
    fc_b = np.asarray(fc_b, dtype=np.float32)

    in_maps = _prepare_in_maps(x, phases, fc_w, fc_b)
    runner = _cached_runner()
    out_by_core = runner(in_maps)
    out = np.concatenate(out_by_core, axis=0)
    return out.astype(np.float32)


def _cached_runner(_cache={}):
    """Build (once) a donated sharded jit wrapper around the Bass module."""
    if "fn" in _cache:
        return _cache["fn"]
    import jax
    import concourse.mybir as _mybir
    from concourse import bass2jax
    from jax.sharding import Mesh, PartitionSpec
    from jax.experimental.shard_map import shard_map

    nc, _ = _build()
    bass2jax.install_neuronx_cc_hook()
    pname = nc.partition_id_tensor.name if nc.partition_id_tensor else None
    in_names, out_names, out_avals = [], [], []
    for alloc in nc.m.functions[0].allocations:
        if not isinstance(alloc, _mybir.MemoryLocationSet):
            continue
        name = alloc.memorylocations[0].name
        if alloc.kind == "ExternalInput":
            if name != pname:
                in_names.append(name)
        elif alloc.kind == "ExternalOutput":
            out_names.append(name)
            out_avals.append(jax.core.ShapedArray(
                tuple(alloc.tensor_shape), _mybir.dt.np(alloc.dtype)))
    n_params = len(in_names)
    all_in = in_names + out_names + ([pname] if pname else [])

    def _body(*args):
        ops = list(args)
        if pname:
            ops.append(bass2jax.partition_id_tensor())
        return tuple(bass2jax._bass_exec_p.bind(
            *ops, out_avals=tuple(out_avals), in_names=tuple(all_in),
            out_names=tuple(out_names), lowering_input_output_aliases=(),
            sim_require_finite=True, sim_require_nnan=True, nc=nc))

    mesh = Mesh(np.asarray(jax.devices()[:N_CORES]), ("core",))
    n_outs = len(out_names)
    sharded = jax.jit(
        shard_map(_body, mesh=mesh,
                  in_specs=(PartitionSpec("core"),) * (n_params + n_outs),
                  out_specs=(PartitionSpec("core"),) * n_outs,
                  check_rep=False),
        donate_argnums=tuple(range(n_params, n_params + n_outs)),
        keep_unused=True,
    )

    def run(in_maps):
        concat_in = [
            np.concatenate([np.asarray(in_maps[c][nm]) for c in range(N_CORES)],
                           axis=0)
            for nm in in_names
        ]
        zeros = [np.zeros((N_CORES * av.shape[0], *av.shape[1:]), av.dtype)
                 for av in out_avals]
        outs = sharded(*concat_in, *zeros)
        oi = out_names.index("out")
        full = np.asarray(outs[oi]).reshape(N_CORES, *out_avals[oi].shape)
        return [full[c] for c in range(N_CORES)]

    _cache["fn"] = run
    return run


def _np_mdt():
    import concourse.mybir as _mybir
    return _mybir.dt.np(MDT)


def _const_arrays(phases, fc_w, fc_b, _cache={}):
    """Host-side constant prep, cached on content (weights rarely change)."""
    import hashlib
    key = hashlib.sha1(phases.tobytes()).hexdigest() + \
        hashlib.sha1(fc_w.tobytes()).hexdigest() + \
        hashlib.sha1(fc_b.tobytes()).hexdigest()
    if _cache.get("key") == key:
        return _cache["val"]
    import concourse.mybir as _mybir
    np_mdt = _np_mdt()
    np_pdt = (np.float32 if os.environ.get("DONN_PMDT") == "f32" else np_mdt)
    wre, wim, wimn, wsum = [a.astype(np_mdt) for a in _host_constants()]
    # device masks only for layers 1.. (layer 0 folded into v0 on host)
    ph = phases.reshape(NL, RB, 128, RES).transpose(0, 2, 1, 3)
    pcos = np.ascontiguousarray(np.cos(ph[1:])).astype(np_pdt)
    psin = np.ascontiguousarray(np.sin(ph[1:])).astype(np_pdt)
    fcw = np.ascontiguousarray(
        fc_w.reshape(NCLS, RB, 128, RES).transpose(2, 0, 1, 3).reshape(128, NCLS, FDIM)
    ).astype(_mybir.dt.np(bf16))
    fcb_rep = np.ascontiguousarray(np.broadcast_to(fc_b[None, :], (PER_CORE, NCLS)))
    val = {"wre": wre, "wim": wim, "wimn": wimn, "wsum": wsum,
           "pcos": pcos, "psin": psin, "fcw": fcw, "fcb": fcb_rep,
           "c0": np.cos(ph[0]).astype(np.float32),
           "s0": np.sin(ph[0]).astype(np.float32)}
    _cache["key"] = key
    _cache["val"] = val
    return val


def _prepare_in_maps(x, phases, fc_w, fc_b):
    consts = _const_arrays(phases, fc_w, fc_b)
    xs = x[:, 0].reshape(x.shape[0], RB, 128, RES).transpose(0, 2, 1, 3)
    np_mdt = _np_mdt()
    c0, s0 = consts["c0"], consts["s0"]
    send = {k: v for k, v in consts.items()
            if k not in ("c0", "s0", "wimn" if ALG == "3m" else "wsum")}
    in_maps = []
    for c in range(N_CORES):
        shard = xs[c * PER_CORE:(c + 1) * PER_CORE]  # [img, 128, RB, RES] f32
        vre = shard * c0
        vim = shard * s0
        planes = [vre, vim] + ([vre + vim] if ALG == "3m" else [])
        v0 = np.ascontiguousarray(
            np.stack(planes, axis=1)).astype(np_mdt)
        in_maps.append({"v0": v0, **send})
    return in_maps


def time_device(inputs, reps=20):
    """Wall-clock the sharded PJRT executable with device-resident inputs.

    Returns the best per-call time in ns (includes dispatch overhead, so an
    upper bound on HW exec time).
    """
    import time as _time
    import jax
    import concourse.mybir as _mybir
    from concourse import bass2jax
    from jax.sharding import Mesh, PartitionSpec, NamedSharding
    from jax.experimental.shard_map import shard_map

    x = np.asarray(inputs["x"], dtype=np.float32)
    in_maps = _prepare_in_maps(
        x, np.asarray(inputs["phases"], np.float32),
        np.asarray(inputs["fc_w"], np.float32),
        np.asarray(inputs["fc_b"], np.float32))

    nc, _ = _build()
    bass2jax.install_neuronx_cc_hook()
    partition_name = nc.partition_id_tensor.name if nc.partition_id_tensor else None

    in_names, out_names, out_avals = [], [], []
    for alloc in nc.m.functions[0].allocations:
        if not isinstance(alloc, _mybir.MemoryLocationSet):
            continue
        name = alloc.memorylocations[0].name
        if alloc.kind == "ExternalInput":
            if name != partition_name:
                in_names.append(name)
        elif alloc.kind == "ExternalOutput":
            out_names.append(name)
            out_avals.append(jax.core.ShapedArray(
                tuple(alloc.tensor_shape), _mybir.dt.np(alloc.dtype)))
    n_params = len(in_names)
    all_in_names = in_names + out_names
    if partition_name is not None:
        all_in_names = all_in_names + [partition_name]

    def _body(*args):
        operands = list(args)
        if partition_name is not None:
            operands.append(bass2jax.partition_id_tensor())
        outs = bass2jax._bass_exec_p.bind(
            *operands,
            out_avals=tuple(out_avals),
            in_names=tuple(all_in_names),
            out_names=tuple(out_names),
            lowering_input_output_aliases=(),
            sim_require_finite=True,
            sim_require_nnan=True,
            nc=nc,
        )
        return tuple(outs)

    devices = jax.devices()[:N_CORES]
    mesh = Mesh(np.asarray(devices), ("core",))
    n_outs = len(out_names)
    in_specs = (PartitionSpec("core"),) * (n_params + n_outs)
    out_specs = (PartitionSpec("core"),) * n_outs
    sharded = jax.jit(
        shard_map(_body, mesh=mesh, in_specs=in_specs, out_specs=out_specs,
                  check_rep=False),
        donate_argnums=tuple(range(n_params, n_params + n_outs)),
        keep_unused=True,
    )
    sh = NamedSharding(mesh, PartitionSpec("core"))
    concat_in = [
        jax.device_put(
            np.concatenate([np.asarray(in_maps[c][nm]) for c in range(N_CORES)], axis=0),
            sh)
        for nm in in_names
    ]
    zero_np = [np.zeros((N_CORES * av.shape[0], *av.shape[1:]), av.dtype)
               for av in out_avals]

    def one_call():
        return sharded(*concat_in, *[jax.device_put(z, sh) for z in zero_np])

    # warmup + sanity: output must be nonzero
    w = one_call()
    jax.block_until_ready(w)
    assert float(np.abs(np.asarray(w[0])).max()) > 0.0, "kernel produced zeros"

    def run_async(k):
        t0 = _time.perf_counter()
        outs = [one_call() for _ in range(k)]
        jax.block_until_ready(outs)
        return _time.perf_counter() - t0

    # min-of-n at several batch sizes, then least-squares slope: robust to
    # the axon tunnel's large positive latency outliers.
    ks = [4, 54, 104]
    mins = []
    for k in ks:
        mins.append(min(run_async(k) for _ in range(6)))
    ks_a = np.asarray(ks, dtype=np.float64)
    ms_a = np.asarray(mins, dtype=np.float64)
    slope = float(np.polyfit(ks_a, ms_a, 1)[0])
    return slope * 1e9


# revision 84
# speedup vs baseline: 1.2241x; 1.2241x over previous
"""Trainium2 Bass kernel for CoherentDONN (3-layer diffractive optical NN).

Math: per layer, field update is
    U' = ifft2(H * fft2(U * exp(i*phi_l)))
H is separable (H = e^{ikz} h x h, Fresnel chirp), so the whole linear step
collapses to  U' = A V A^T  with  A = conj(F) diag(h) F / 512  (circulant,
unitary).  e^{ikz} is unit-modulus and drops out of the final intensity.
On the PE (out = lhsT.T @ rhs) with W = A^T:
    S = V^T W      (= (A V)^T)
    U' = S^T W     (= A V A^T)
so each layer is exactly two 512-contraction complex matmul stages with the
same moving operand W and zero transposes/elementwise H work.

Each complex matmul stage uses the 3M (Karatsuba) decomposition:
    P1 = Vre Wre,  P2 = Vim Wim,  P3 = (Vre+Vim)(Wre+Wim)
    Sre = P1 - P2,  Sim = P3 - P1 - P2
i.e. 12 accumulating 128x128x512 matmuls per 128-row output block instead of
the schoolbook 16 (bf16 operands run the PE at full rate and rel-err stays
~1.2e-2, well inside the 2e-2 gate).

Engine layout (engines read at most ONE PSUM operand per instruction): the
Act engine drains each PSUM product to SBUF (each copy hides under the next
matmul chain, so PSUM ring slots free ~1us after their chain); the DVE does
all combines/phase-rotation as all-16-bit SBUF ops in its 2x mode; Pool
(gpsimd) takes slack-tolerant plane sums; layer-0's phase multiply is folded
into the host-prepared input planes.  Two images are kept in flight so every
producer has a full stage of latency cover; the vim/vsum half of each phase
rotation is emitted one stage deferred to keep the DVE inside the layer-
boundary window budget.  TimelineSim makespan: ~1.028 ms/core (PE ~97% busy
at the 12-matmul/block floor).

Sharding: pure data parallelism, 16 images per core across 8 cores.
"""

import os
import numpy as np

import concourse.bass as bass
import concourse.mybir as mybir
import concourse.tile as tile
from concourse import bacc
from concourse.bass_utils import run_bass_kernel_spmd

N_CORES = int(os.environ.get("DONN_CORES", "8"))
PER_CORE = int(os.environ.get("DONN_IMG", str(128 // max(N_CORES, 1))))
RES = 512
NL = 3
NCLS = 10
RB = RES // 128            # 4 row blocks of 128 partitions
FDIM = RES * RES // 128    # 2048 feat positions per partition
FC_BLK = 256               # feat positions per FC chunk
LAMBDA = 5.32e-07
Z = 0.035
DX = 1e-06

ALG = os.environ.get("DONN_ALG", "3m")   # "3m" | "school"

f32 = mybir.dt.float32
f32r = mybir.dt.float32r
bf16 = mybir.dt.bfloat16
_mdt_default = "bf16" if ALG == "3m" else "f32r"
MDT = {"f32r": f32r, "bf16": bf16, "f32": f32}[
    os.environ.get("DONN_MMDT", _mdt_default)]
MULT = mybir.AluOpType.mult
ADD = mybir.AluOpType.add
SUB = mybir.AluOpType.subtract
SQUARE = mybir.ActivationFunctionType.Square
COPY = mybir.ActivationFunctionType.Copy


def _host_constants():
    fx = np.fft.fftfreq(RES, DX)
    h = np.exp(-1j * np.pi * LAMBDA * Z * fx**2)
    a = np.fft.ifft(h)
    idx = (np.arange(RES)[:, None] - np.arange(RES)[None, :]) % RES
    W = a[idx].T.copy()  # W = A^T, complex128
    # [row k, col n] -> [p, c, n] with k = c*128 + p
    def lay(m):
        return np.ascontiguousarray(
            m.reshape(RB, 128, RES).transpose(1, 0, 2).astype(np.float32)
        )
    return (lay(W.real), lay(W.imag), lay(-W.imag), lay(W.real + W.imag))


def _build(nc_handle_cache={}):
    if "nc" in nc_handle_cache:
        return nc_handle_cache["nc"], nc_handle_cache["aps"]

    nc = bacc.Bacc("TRN2", target_bir_lowering=False, debug=False,
                   num_devices=N_CORES)

    # layer-0 phase multiply is precomputed on host: per image the planes
    # (x*cos(phi0), x*sin(phi0)[, x*(cos+sin)]) ship as MDT directly.
    NV = 3 if ALG == "3m" else 2
    x_d = nc.dram_tensor("v0", [PER_CORE, NV, 128, RB, RES], MDT,
                         kind="ExternalInput").ap()
    PDT = f32 if os.environ.get("DONN_PMDT") == "f32" else MDT
    wre_d = nc.dram_tensor("wre", [128, RB, RES], MDT, kind="ExternalInput").ap()
    wim_d = nc.dram_tensor("wim", [128, RB, RES], MDT, kind="ExternalInput").ap()
    if ALG == "3m":
        wx_d = nc.dram_tensor("wsum", [128, RB, RES], MDT, kind="ExternalInput").ap()
    else:
        wx_d = nc.dram_tensor("wimn", [128, RB, RES], MDT, kind="ExternalInput").ap()
    pc_d = nc.dram_tensor("pcos", [NL - 1, 128, RB, RES], PDT,
                          kind="ExternalInput").ap()
    psn_d = nc.dram_tensor("psin", [NL - 1, 128, RB, RES], PDT,
                           kind="ExternalInput").ap()
    fcw_d = nc.dram_tensor("fcw", [128, NCLS, FDIM], bf16, kind="ExternalInput").ap()
    fcb_d = nc.dram_tensor("fcb", [PER_CORE, NCLS], f32, kind="ExternalInput").ap()
    out_d = nc.dram_tensor("out", [PER_CORE, NCLS], f32, kind="ExternalOutput").ap()

    with tile.TileContext(nc) as tc:
        with tc.tile_pool(name="consts", bufs=1) as constp, \
             tc.tile_pool(name="dram", bufs=1, space="DRAM") as dramp:
            featbuf = dramp.tile([PER_CORE, 128, FDIM], bf16)

            # W planes and phase masks arrive pre-converted to MDT/PDT from
            # the host: no staging, no convert, minimal startup DMA.
            w_tiles = {}
            for name, src in (("wre", wre_d), ("wim", wim_d), ("wx", wx_d)):
                wt = constp.tile([128, RB, RES], MDT, tag=name)
                nc.sync.dma_start(wt[:], src[:])
                w_tiles[name] = wt
            wre, wim, wx = w_tiles["wre"], w_tiles["wim"], w_tiles["wx"]

            pcos, psin = [None], [None]
            for l in range(1, NL):
                for src, lst, nm in ((pc_d, pcos, "pc"), (psn_d, psin, "ps")):
                    ct = constp.tile([128, RB, RES], PDT, tag=f"{nm}{l}")
                    nc.sync.dma_start(ct[:], src[l - 1])
                    lst.append(ct)

            fcb_t = constp.tile([PER_CORE, NCLS], f32, tag="fcb")
            nc.sync.dma_start(fcb_t[:], fcb_d[:])

            with tc.tile_pool(name="vp", bufs=4) as vpool, \
                 tc.tile_pool(name="up", bufs=4) as upool, \
                 tc.tile_pool(name="sp", bufs=2) as spool, \
                 tc.tile_pool(name="tp", bufs=3) as tpool, \
                 tc.tile_pool(name="fp", bufs=3) as fpool, \
                 tc.tile_pool(name="ps", bufs=4 if ALG == "3m" else 8,
                              space="PSUM") as psum:

                # ---------------- 3M (Karatsuba) path ----------------
                def load_and_pm1_3m(i):
                    # scalar-engine DMA queue: image loads bypass the sync
                    # queue that carries W staging and feature writebacks
                    vre = vpool.tile([128, RB, RES], MDT, tag="vre")
                    vim = vpool.tile([128, RB, RES], MDT, tag="vim")
                    vsm = vpool.tile([128, RB, RES], MDT, tag="vsm")
                    nc.scalar.dma_start(vre[:], x_d[i, 0])
                    nc.scalar.dma_start(vim[:], x_d[i, 1])
                    nc.scalar.dma_start(vsm[:], x_d[i, 2])
                    return vre, vim, vsm

                def mm_stage_3m(lre, lim, lsm, to_sbuf):
                    """One complex stage via 3M.  PSUM tiles span TWO row
                    blocks (2 banks); the Act engine drains each product to
                    SBUF and the DVE combines the SBUF copies.  If to_sbuf,
                    produce MDT planes (sre, sim, ssum) for the next stage;
                    else produce per-pair (ure, uim) for phase rotation or
                    intensity."""
                    if to_sbuf:
                        sre = spool.tile([128, RB, RES], MDT, tag="sre")
                        sim = spool.tile([128, RB, RES], MDT, tag="sim")
                        ssm = spool.tile([128, RB, RES], MDT, tag="ssm")

                    def chain(ptile, h, mi, lhs, w):
                        ms = bass.ts(mi, 128)
                        for c in range(RB):
                            nc.tensor.matmul(ptile[:, h, :], lhs[:, c, ms],
                                             w[:, c, :],
                                             start=(c == 0), stop=(c == RB - 1))

                    # Two row-block pairs share the 8-bank ring (4 tiles x 2
                    # banks).  Each product's Act copy is its only PSUM
                    # reader and hides under the next matmul chain, so ring
                    # slots free ~1us after their chain and the PE never
                    # waits on a PSUM slot.
                    # Engines read at most ONE PSUM input per instruction,
                    # so each product is drained psum->SBUF by the idle Act
                    # engine (every copy hides under the next matmul chain
                    # and frees its ring slot ~1us after the chain ends).
                    # All combines then run as all-16-bit SBUF DVE ops at
                    # the 2x rate, fully decoupled from the PSUM ring.
                    out = [None] * (RB // 2)
                    for mp in range(RB // 2):
                        p1 = psum.tile([128, 2, RES], f32, tag="st", name="p1")
                        for h in range(2):
                            chain(p1, h, 2 * mp + h, lre, wre)
                        cp1 = tpool.tile([128, 2, RES], MDT, tag="cp1")
                        nc.scalar.activation(cp1[:], p1[:], COPY)
                        p2 = psum.tile([128, 2, RES], f32, tag="st", name="p2")
                        for h in range(2):
                            chain(p2, h, 2 * mp + h, lim, wim)
                        cp2 = tpool.tile([128, 2, RES], MDT, tag="cp2")
                        nc.scalar.activation(cp2[:], p2[:], COPY)
                        mslc = bass.ts(mp, 2)
                        u = tpool.tile([128, 2, RES], MDT, tag="u", name="u")
                        if to_sbuf:
                            nc.vector.tensor_tensor(sre[:, mslc, :], cp1[:],
                                                    cp2[:], SUB)
                        else:
                            ure = upool.tile([128, 2, RES], MDT, tag="ure")
                            nc.vector.tensor_tensor(ure[:], cp1[:], cp2[:], SUB)
                        nc.vector.tensor_tensor(u[:], cp1[:], cp2[:], ADD)
                        p3 = psum.tile([128, 2, RES], f32, tag="st", name="p3")
                        for h in range(2):
                            chain(p3, h, 2 * mp + h, lsm, wx)
                        cp3 = tpool.tile([128, 2, RES], MDT, tag="cp3")
                        nc.scalar.activation(cp3[:], p3[:], COPY)
                        if to_sbuf:
                            nc.vector.tensor_tensor(sim[:, mslc, :], cp3[:],
                                                    u[:], SUB)
                            # ssm's consumer (next stage's P3 chain) is ~3
                            # chains away: Pool's latency is fine, and this
                            # keeps the DVE queue free of head-of-line work
                            nc.gpsimd.tensor_tensor(ssm[:, mslc, :],
                                                    sre[:, mslc, :],
                                                    sim[:, mslc, :], ADD)
                        else:
                            uim = upool.tile([128, 2, RES], MDT, tag="uim")
                            nc.vector.tensor_tensor(uim[:], cp3[:], u[:], SUB)
                            out[mp] = (ure, uim)
                    if to_sbuf:
                        return sre, sim, ssm
                    return out

                def pm_from_psum_3m(l, us):
                    """V_l = U_{l-1} * exp(i*phi_l) from per-pair (ure, uim).

                    The vre path (needed by the next stage's first chains)
                    is emitted inline on the DVE.  The vim/vsm path is
                    returned as a deferred closure: it's consumed two stage
                    slots later, so the caller emits it after the paired
                    image's stage — keeping the DVE queue inside the layer-
                    boundary window budget."""
                    vre = vpool.tile([128, RB, RES], MDT, tag="vre")
                    vim = vpool.tile([128, RB, RES], MDT, tag="vim")
                    vsm = vpool.tile([128, RB, RES], MDT, tag="vsm")
                    for mp, (ure, uim) in enumerate(us):
                        mslc = bass.ts(mp, 2)
                        c_ap = pcos[l][:, mslc, :]
                        s_ap = psin[l][:, mslc, :]
                        t1 = upool.tile([128, 2, RES], MDT, tag="t1")
                        t2 = upool.tile([128, 2, RES], MDT, tag="t2")
                        nc.vector.tensor_tensor(t1[:], ure[:], c_ap, MULT)
                        nc.vector.tensor_tensor(t2[:], uim[:], s_ap, MULT)
                        nc.vector.tensor_tensor(vre[:, mslc, :], t1[:], t2[:], SUB)

                    def finish_vim():
                        for mp, (ure, uim) in enumerate(us):
                            mslc = bass.ts(mp, 2)
                            c_ap = pcos[l][:, mslc, :]
                            s_ap = psin[l][:, mslc, :]
                            t3 = upool.tile([128, 2, RES], MDT, tag="t3")
                            t4 = upool.tile([128, 2, RES], MDT, tag="t4")
                            nc.gpsimd.tensor_tensor(t3[:], ure[:], s_ap, MULT)
                            nc.vector.tensor_tensor(t4[:], uim[:], c_ap, MULT)
                            nc.vector.tensor_tensor(vim[:, mslc, :],
                                                    t3[:], t4[:], ADD)
                            nc.gpsimd.tensor_tensor(vsm[:, mslc, :],
                                                    vre[:, mslc, :],
                                                    vim[:, mslc, :], ADD)
                    return (vre, vim, vsm), finish_vim

                def intensity_3m(i, us):
                    # split squares across Act/DVE and keep the ft add on
                    # the DVE: this chain gates the FC readback at the very
                    # end of the kernel, so its serial latency matters
                    for mp, (ure, uim) in enumerate(us):
                        s0 = tpool.tile([128, 2, RES], bf16, tag="s0")
                        s1 = tpool.tile([128, 2, RES], bf16, tag="s1")
                        nc.scalar.activation(s0[:], ure[:], SQUARE)
                        nc.vector.tensor_tensor(s1[:], uim[:], uim[:], MULT)
                        ft = fpool.tile([128, 2, RES], bf16, tag="ft")
                        nc.vector.tensor_tensor(ft[:], s0[:], s1[:], ADD)
                        nc.sync.dma_start(
                            featbuf[i, :, bass.ts(mp, 2 * RES)], ft[:])

                # ---------------- schoolbook path (A/B reference) --------
                def load_and_pm1(i):
                    vre = vpool.tile([128, RB, RES], MDT, tag="vre")
                    vim = vpool.tile([128, RB, RES], MDT, tag="vim")
                    nc.sync.dma_start(vre[:], x_d[i, 0])
                    nc.sync.dma_start(vim[:], x_d[i, 1])
                    return vre, vim

                def mm_stage(lre, lim, to_sbuf):
                    if to_sbuf:
                        sre = spool.tile([128, RB, RES], MDT, tag="sre")
                        sim = spool.tile([128, RB, RES], MDT, tag="sim")
                    ps_pairs = []
                    for m in range(RB):
                        ms = bass.ts(m, 128)
                        pr = psum.tile([128, RES], f32, tag="st")
                        pi = psum.tile([128, RES], f32, tag="st")
                        for c in range(RB):
                            nc.tensor.matmul(pr[:], lre[:, c, ms], wre[:, c, :],
                                             start=(c == 0), stop=False)
                        for c in range(RB):
                            nc.tensor.matmul(pr[:], lim[:, c, ms], wx[:, c, :],
                                             start=False, stop=(c == RB - 1))
                        for c in range(RB):
                            nc.tensor.matmul(pi[:], lre[:, c, ms], wim[:, c, :],
                                             start=(c == 0), stop=False)
                        for c in range(RB):
                            nc.tensor.matmul(pi[:], lim[:, c, ms], wre[:, c, :],
                                             start=False, stop=(c == RB - 1))
                        if to_sbuf:
                            nc.vector.tensor_copy(sre[:, m, :], pr[:])
                            nc.scalar.activation(sim[:, m, :], pi[:], COPY)
                        else:
                            ps_pairs.append((pr, pi))
                    if to_sbuf:
                        return sre, sim
                    return ps_pairs

                def pm_from_psum(l, ps_pairs):
                    vre = vpool.tile([128, RB, RES], MDT, tag="vre")
                    vim = vpool.tile([128, RB, RES], MDT, tag="vim")
                    for m, (pr, pi) in enumerate(ps_pairs):
                        c_ap = pcos[l][:, m, :]
                        s_ap = psin[l][:, m, :]
                        t1 = tpool.tile([128, RES], f32, tag="t")
                        t2 = tpool.tile([128, RES], f32, tag="t")
                        nc.vector.tensor_tensor(t1[:], pr[:], c_ap, MULT)
                        nc.vector.tensor_tensor(t2[:], pi[:], s_ap, MULT)
                        nc.vector.tensor_tensor(vre[:, m, :], t1[:], t2[:], SUB)
                        t3 = tpool.tile([128, RES], f32, tag="t")
                        t4 = tpool.tile([128, RES], f32, tag="t")
                        nc.vector.tensor_tensor(t3[:], pr[:], s_ap, MULT)
                        nc.vector.tensor_tensor(t4[:], pi[:], c_ap, MULT)
                        nc.vector.tensor_tensor(vim[:, m, :], t3[:], t4[:], ADD)
                    return vre, vim

                def intensity(i, ps_pairs):
                    for m, (pr, pi) in enumerate(ps_pairs):
                        s0 = tpool.tile([128, RES], f32, tag="t")
                        s1 = tpool.tile([128, RES], f32, tag="t")
                        nc.scalar.activation(s0[:], pr[:], SQUARE)
                        nc.scalar.activation(s1[:], pi[:], SQUARE)
                        ft = fpool.tile([128, RES], bf16, tag="ft")
                        nc.vector.tensor_tensor(ft[:], s0[:], s1[:], ADD)
                        nc.sync.dma_start(featbuf[i, :, bass.ts(m, RES)], ft[:])

                if ALG == "3m":
                    f_load, f_stage, f_pm, f_int = (
                        load_and_pm1_3m, mm_stage_3m, pm_from_psum_3m,
                        intensity_3m)
                else:
                    f_load, f_stage, f_pm, f_int = (
                        load_and_pm1, mm_stage, pm_from_psum, intensity)

                npair = (PER_CORE + 1) // 2
                vcur = {}
                vcur[0] = f_load(0)
                if PER_CORE > 1:
                    vcur[1] = f_load(1)
                for pr_i in range(npair):
                    imgs = [i for i in (2 * pr_i, 2 * pr_i + 1) if i < PER_CORE]
                    for l in range(NL):
                        s_tiles = {}
                        for i in imgs:
                            s_tiles[i] = f_stage(*vcur[i], to_sbuf=True)
                        deferred = []
                        for i in imgs:
                            ps = f_stage(*s_tiles[i], to_sbuf=False)
                            if l < NL - 1:
                                if ALG == "3m":
                                    vcur[i], fin = f_pm(l + 1, ps)
                                    deferred.append(fin)
                                else:
                                    vcur[i] = f_pm(l + 1, ps)
                            else:
                                f_int(i, ps)
                        for fin in deferred:
                            fin()
                        if l == 0:
                            for i_next in (2 * pr_i + 2, 2 * pr_i + 3):
                                if i_next < PER_CORE:
                                    vcur[i_next] = f_load(i_next)

            # ---- FC over all images ----
            # wch chunks are static weights: preload them all on a separate
            # queue so the tail is only fch readback + matmul.
            nblk = FDIM // FC_BLK
            with tc.tile_pool(name="fcw", bufs=nblk) as fwpool, \
                 tc.tile_pool(name="fcp", bufs=8) as fcpool, \
                 tc.tile_pool(name="fps", bufs=1, space="PSUM") as fpsum:
                wchs = []
                for blk in range(nblk):
                    wch = fwpool.tile([128, NCLS, FC_BLK], bf16, tag="wch")
                    nc.gpsimd.dma_start(wch[:], fcw_d[:, :, bass.ts(blk, FC_BLK)])
                    wchs.append(wch)
                ps_fc = fpsum.tile([PER_CORE, NCLS], f32, tag="fc")
                feat_t = featbuf[:].rearrange("i p f -> p i f")
                fc_q = [nc.sync, nc.scalar, nc.gpsimd]
                for blk in range(nblk):
                    fs = bass.ts(blk, FC_BLK)
                    fch = fcpool.tile([128, PER_CORE, FC_BLK], bf16, tag="fch")
                    # rotate DMA queues so chunk readbacks overlap
                    fc_q[blk % 3].dma_start(fch[:], feat_t[:, :, fs])
                    for j in range(FC_BLK):
                        nc.tensor.matmul(ps_fc[:], fch[:, :, j], wchs[blk][:, :, j],
                                         start=(blk == 0 and j == 0),
                                         stop=(blk == nblk - 1 and j == FC_BLK - 1))
                out_sb = fcpool.tile([PER_CORE, NCLS], f32, tag="osb")
                nc.vector.tensor_tensor(out_sb[:], ps_fc[:], fcb_t[:], ADD)
                nc.sync.dma_start(out_d[:], out_sb[:])

    nc.compile()
    aps = None
    nc_handle_cache["nc"] = nc
    nc_handle_cache["aps"] = aps
    return nc, aps


def kernel(x, phases, fc_w, fc_b):
    x = np.asarray(x, dtype=np.float32)
    phases = np.asarray(phases, dtype=np.float32)
    fc_w = np.asarray(fc_w, dtype=np.float32)
    fc_b = np.asarray(fc_b, dtype=np.float32)

    in_maps = _prepare_in_maps(x, phases, fc_w, fc_b)
    runner = _cached_runner()
    out_by_core = runner(in_maps)
    out = np.concatenate(out_by_core, axis=0)
    return out.astype(np.float32)


def _cached_runner(_cache={}):
    """Build (once) a donated sharded jit wrapper around the Bass module."""
    if "fn" in _cache:
        return _cache["fn"]
    import jax
    import concourse.mybir as _mybir
    from concourse import bass2jax
    from jax.sharding import Mesh, PartitionSpec
    from jax.experimental.shard_map import shard_map

    nc, _ = _build()
    bass2jax.install_neuronx_cc_hook()
    pname = nc.partition_id_tensor.name if nc.partition_id_tensor else None
    in_names, out_names, out_avals = [], [], []
    for alloc in nc.m.functions[0].allocations:
        if not isinstance(alloc, _mybir.MemoryLocationSet):
            continue
        name = alloc.memorylocations[0].name
        if alloc.kind == "ExternalInput":
            if name != pname:
                in_names.append(name)
        elif alloc.kind == "ExternalOutput":
            out_names.append(name)
            out_avals.append(jax.core.ShapedArray(
                tuple(alloc.tensor_shape), _mybir.dt.np(alloc.dtype)))
    n_params = len(in_names)
    all_in = in_names + out_names + ([pname] if pname else [])

    def _body(*args):
        ops = list(args)
        if pname:
            ops.append(bass2jax.partition_id_tensor())
        return tuple(bass2jax._bass_exec_p.bind(
            *ops, out_avals=tuple(out_avals), in_names=tuple(all_in),
            out_names=tuple(out_names), lowering_input_output_aliases=(),
            sim_require_finite=True, sim_require_nnan=True, nc=nc))

    mesh = Mesh(np.asarray(jax.devices()[:N_CORES]), ("core",))
    n_outs = len(out_names)
    sharded = jax.jit(
        shard_map(_body, mesh=mesh,
                  in_specs=(PartitionSpec("core"),) * (n_params + n_outs),
                  out_specs=(PartitionSpec("core"),) * n_outs,
                  check_rep=False),
        donate_argnums=tuple(range(n_params, n_params + n_outs)),
        keep_unused=True,
    )

    def run(in_maps):
        concat_in = [
            np.concatenate([np.asarray(in_maps[c][nm]) for c in range(N_CORES)],
                           axis=0)
            for nm in in_names
        ]
        zeros = [np.zeros((N_CORES * av.shape[0], *av.shape[1:]), av.dtype)
                 for av in out_avals]
        outs = sharded(*concat_in, *zeros)
        oi = out_names.index("out")
        full = np.asarray(outs[oi]).reshape(N_CORES, *out_avals[oi].shape)
        return [full[c] for c in range(N_CORES)]

    _cache["fn"] = run
    return run


def _np_mdt():
    import concourse.mybir as _mybir
    return _mybir.dt.np(MDT)


def _const_arrays(phases, fc_w, fc_b, _cache={}):
    """Host-side constant prep, cached on content (weights rarely change)."""
    import hashlib
    key = hashlib.sha1(phases.tobytes()).hexdigest() + \
        hashlib.sha1(fc_w.tobytes()).hexdigest() + \
        hashlib.sha1(fc_b.tobytes()).hexdigest()
    if _cache.get("key") == key:
        return _cache["val"]
    import concourse.mybir as _mybir
    np_mdt = _np_mdt()
    np_pdt = (np.float32 if os.environ.get("DONN_PMDT") == "f32" else np_mdt)
    wre, wim, wimn, wsum = [a.astype(np_mdt) for a in _host_constants()]
    # device masks only for layers 1.. (layer 0 folded into v0 on host)
    ph = phases.reshape(NL, RB, 128, RES).transpose(0, 2, 1, 3)
    pcos = np.ascontiguousarray(np.cos(ph[1:])).astype(np_pdt)
    psin = np.ascontiguousarray(np.sin(ph[1:])).astype(np_pdt)
    fcw = np.ascontiguousarray(
        fc_w.reshape(NCLS, RB, 128, RES).transpose(2, 0, 1, 3).reshape(128, NCLS, FDIM)
    ).astype(_mybir.dt.np(bf16))
    fcb_rep = np.ascontiguousarray(np.broadcast_to(fc_b[None, :], (PER_CORE, NCLS)))
    val = {"wre": wre, "wim": wim, "wimn": wimn, "wsum": wsum,
           "pcos": pcos, "psin": psin, "fcw": fcw, "fcb": fcb_rep,
           "c0": np.cos(ph[0]).astype(np.float32),
           "s0": np.sin(ph[0]).astype(np.float32)}
    _cache["key"] = key
    _cache["val"] = val
    return val


def _prepare_in_maps(x, phases, fc_w, fc_b):
    consts = _const_arrays(phases, fc_w, fc_b)
    xs = x[:, 0].reshape(x.shape[0], RB, 128, RES).transpose(0, 2, 1, 3)
    np_mdt = _np_mdt()
    c0, s0 = consts["c0"], consts["s0"]
    send = {k: v for k, v in consts.items()
            if k not in ("c0", "s0", "wimn" if ALG == "3m" else "wsum")}
    in_maps = []
    for c in range(N_CORES):
        shard = xs[c * PER_CORE:(c + 1) * PER_CORE]  # [img, 128, RB, RES] f32
        vre = shard * c0
        vim = shard * s0
        planes = [vre, vim] + ([vre + vim] if ALG == "3m" else [])
        v0 = np.ascontiguousarray(
            np.stack(planes, axis=1)).astype(np_mdt)
        in_maps.append({"v0": v0, **send})
    return in_maps


def time_device(inputs, reps=20):
    """Wall-clock the sharded PJRT executable with device-resident inputs.

    Returns the best per-call time in ns (includes dispatch overhead, so an
    upper bound on HW exec time).
    """
    import time as _time
    import jax
    import concourse.mybir as _mybir
    from concourse import bass2jax
    from jax.sharding import Mesh, PartitionSpec, NamedSharding
    from jax.experimental.shard_map import shard_map

    x = np.asarray(inputs["x"], dtype=np.float32)
    in_maps = _prepare_in_maps(
        x, np.asarray(inputs["phases"], np.float32),
        np.asarray(inputs["fc_w"], np.float32),
        np.asarray(inputs["fc_b"], np.float32))

    nc, _ = _build()
    bass2jax.install_neuronx_cc_hook()
    partition_name = nc.partition_id_tensor.name if nc.partition_id_tensor else None

    in_names, out_names, out_avals = [], [], []
    for alloc in nc.m.functions[0].allocations:
        if not isinstance(alloc, _mybir.MemoryLocationSet):
            continue
        name = alloc.memorylocations[0].name
        if alloc.kind == "ExternalInput":
            if name != partition_name:
                in_names.append(name)
        elif alloc.kind == "ExternalOutput":
            out_names.append(name)
            out_avals.append(jax.core.ShapedArray(
                tuple(alloc.tensor_shape), _mybir.dt.np(alloc.dtype)))
    n_params = len(in_names)
    all_in_names = in_names + out_names
    if partition_name is not None:
        all_in_names = all_in_names + [partition_name]

    def _body(*args):
        operands = list(args)
        if partition_name is not None:
            operands.append(bass2jax.partition_id_tensor())
        outs = bass2jax._bass_exec_p.bind(
            *operands,
            out_avals=tuple(out_avals),
            in_names=tuple(all_in_names),
            out_names=tuple(out_names),
            lowering_input_output_aliases=(),
            sim_require_finite=True,
            sim_require_nnan=True,
            nc=nc,
        )
        return tuple(outs)

    devices = jax.devices()[:N_CORES]
    mesh = Mesh(np.asarray(devices), ("core",))
    n_outs = len(out_names)
    in_specs = (PartitionSpec("core"),) * (n_params + n_outs)
    out_specs = (PartitionSpec("core"),) * n_outs
    sharded = jax.jit(
        shard_map(_body, mesh=mesh, in_specs=in_specs, out_specs=out_specs,
                  check_rep=False),
        donate_argnums=tuple(range(n_params, n_params + n_outs)),
        keep_unused=True,
    )
    sh = NamedSharding(mesh, PartitionSpec("core"))
    concat_in = [
        jax.device_put(
            np.concatenate([np.asarray(in_maps[c][nm]) for c in range(N_CORES)], axis=0),
            sh)
        for nm in in_names
    ]
    zero_np = [np.zeros((N_CORES * av.shape[0], *av.shape[1:]), av.dtype)
               for av in out_avals]

    def one_call():
        return sharded(*concat_in, *[jax.device_put(z, sh) for z in zero_np])

    # warmup + sanity: output must be nonzero
    w = one_call()
    jax.block_until_ready(w)
    assert float(np.abs(np.asarray(w[0])).max()) > 0.0, "kernel produced zeros"

    def run_async(k):
        t0 = _time.perf_counter()
        outs = [one_call() for _ in range(k)]
        jax.block_until_ready(outs)
        return _time.perf_counter() - t0

    # min-of-n at several batch sizes, then least-squares slope: robust to
    # the axon tunnel's large positive latency outliers.
    ks = [4, 54, 104]
    mins = []
    for k in ks:
        mins.append(min(run_async(k) for _ in range(6)))
    ks_a = np.asarray(ks, dtype=np.float64)
    ms_a = np.asarray(mins, dtype=np.float64)
    slope = float(np.polyfit(ks_a, ms_a, 1)[0])
    return slope * 1e9
